# revision 17
# baseline (speedup 1.0000x reference)
"""Trainium2 Bass kernel for nn_CortexNetwork (dense_cnn, memory-bound).

Reference computation:
    patches[c,i,j,u,v] = x[c, rx[i]+u, ry[j]+v]
    aff[i,j] = sum_{c,u,v} patches * Wa
    exc[i,j] = sum_c prev[c,i,j] * sum_{x,y} We[c,i,j,x,y]   (inh likewise, Wi)
    out      = broadcast_c(relu(aff + 0.9*exc - 0.9*inh))

Strategy: tensor-parallel over the 36x36=1296 grid units, 162 units per
core on 8 cores (padded to 168 = 21 groups of 8 so every tile is a full
128 partitions = 16 channels x 8 units); every reduction is unit-local
so there are no collectives.  The kernel is HBM-bandwidth-bound, so all
streamed data is fp8_e4m3, quantized on the host with sum-preserving
rounding:

  * lateral We|-Wi rows (x64 scale) use error-diffusion rounding along
    each row, so the device's row sums match the f32 sums to ~1e-3;
  * afferent weights (x64) are rounded with the running product-sum
    carried against the fp8 patches (x16), GPTQ-style, so the device's
    dot products track the f32 products; patches are plain RTN fp8.

Measured end-to-end max-rel-error on the reference inputs is ~1.8e-3
(vs the 2e-2 gate) at 1/4 the f32 bytes.

The 2592-wide lateral row sums are split across three engines so no
engine exceeds the ~30us DMA stream time.  16 groups go to the tensor
engine: their lateral columns are host-transposed (zero-padded to
21x128) into 128x128 LDWEIGHTS blocks consumed as chained matmuls
against a constant fp8 ones vector, accumulating each group's
per-(c,unit) row sum into one PSUM column.  5 groups go to ScalarE as
activation(Copy, scale=0.9*prev/64, accum_out).  VectorE runs the fused
afferent multiply-reduce (scalar_tensor_tensor) per group, the
0.9*prev/64 multiply for the PE block, and the final relu.  The
16-channel sums are 0/1-selector matmuls on PE (the afferent selector
carries the 1/1024 dequant scale).

Tail packing: the last-streamed group owns output column 20 alone and
its record is sent as two DMAs (afferent first); every other column's
channel sums, relu and output DMA (on the scalar-engine HWDGE ring, so
the sync FIFO stays clear) are emitted right after their dependencies
mid-loop, so after the final DMA lands only one group's matmuls, one
merge, one 1-column channel sum and a 32-byte output write remain.
"""

import numpy as np
import ml_dtypes

import concourse.bass as bass
import concourse.bacc as bacc
import concourse.mybir as mybir
from concourse import tile
from concourse.bass_utils import run_bass_kernel_spmd

N_CORES = 8
C = 16
GX = GY = 36
RF = 24
IMG = 64
GAMMA = 0.9

UNITS = GX * GY                  # 1296
PER_CORE = UNITS // N_CORES      # 162
S = 8                            # units per group (partition dim C*S=128)
T = 21                           # groups per core (168 units, 6 padded)
PADU = T * S                     # 168
FW = GX * GY                     # lateral free size per channel: 1296
FA = RF * RF                     # afferent free size per channel: 576
LCOL = 2 * FW                    # 2592 lateral elems per (c,unit)
KC = 21                          # 128-chunks of the padded lateral dim
LPAD = KC * 128                  # 2688
COLS_A = LCOL + 2 * FA           # 3744  (ScalarE-group record)
COLS_P = LPAD + 2 * FA           # 3840  (PE-group record, lateral transposed)
WSCALE = 64.0                    # fp8 scale for We/Wi/Wa
PSCALE = 16.0                    # fp8 scale for patches
ACT_SET = (2, 6, 10, 14, 16)     # groups whose lateral runs on ScalarE
PE_GROUPS = tuple(t for t in range(T) if t not in ACT_SET)
NPE = len(PE_GROUPS)             # 16
LAST = PE_GROUPS[-1]             # 20
# output columns: PE groups except the last at 0..14, ACT at 15..19, and
# the last group alone at 20 so only that column waits on the final DMA
COL_OF = {t: i for i, t in enumerate(PE_GROUPS[:-1])}
COL_OF.update({t: NPE - 1 + j for j, t in enumerate(ACT_SET)})
COL_OF[LAST] = T - 1
PSCOL_OF = {t: i for i, t in enumerate(PE_GROUPS)}   # pslat column

F8 = ml_dtypes.float8_e4m3

_PROGRAM_CACHE = {}


def _build_program():
    f32 = mybir.dt.float32
    f8 = mybir.dt.float8e4
    bf16 = mybir.dt.bfloat16
    AL = mybir.AluOpType
    AF = mybir.ActivationFunctionType

    nc = bacc.Bacc(
        "TRN2", target_bir_lowering=False, debug=False, num_devices=N_CORES
    )
    bigp_d = nc.dram_tensor("bigp", [NPE, 128, COLS_P], f8, kind="ExternalInput").ap()
    biga_d = nc.dram_tensor("biga", [T - NPE, 128, COLS_A], f8, kind="ExternalInput").ap()
    possb_d = nc.dram_tensor("possb", [128, T], f32, kind="ExternalInput").ap()
    sel_d = nc.dram_tensor("sel", [128, S], f32, kind="ExternalInput").ap()
    sela_d = nc.dram_tensor("sela", [128, S], f32, kind="ExternalInput").ap()
    out_d = nc.dram_tensor("out", [S, T], f32, kind="ExternalOutput").ap()

    with tile.TileContext(nc) as tc:
        with (
            tc.tile_pool(name="wp", bufs=8) as wpp,
            tc.tile_pool(name="wa", bufs=4) as wap,
            tc.tile_pool(name="cst", bufs=1) as cp,
            tc.tile_pool(name="junk", bufs=3) as jp,
            tc.tile_pool(name="fin", bufs=1) as fp,
            tc.tile_pool(name="ps", bufs=1, space="PSUM") as pp,
        ):
            possb = cp.tile([128, T], f32, tag="possb")
            sel = cp.tile([128, S], f32, tag="sel")
            sela = cp.tile([128, S], f32, tag="sela")
            ones = cp.tile([128, 1], f8, tag="ones")
            plat = cp.tile([128, T], f32, tag="plat")
            paff = cp.tile([128, T], f32, tag="paff")
            nc.gpsimd.dma_start(possb[:], possb_d[:])
            nc.gpsimd.dma_start(sel[:], sel_d[:])
            nc.gpsimd.dma_start(sela[:], sela_d[:])
            nc.vector.memset(ones[:], 1.0)

            wlaff = cp.tile([128, 2 * FA], f8, tag="wlaff")

            pslat = pp.tile([128, NPE], f32, tag="pslat")
            psum = pp.tile([S, T], f32, tag="ps")
            res = fp.tile([S, T], f32, tag="res")

            def chansum(c0, c1):
                nc.tensor.matmul(psum[:, c0:c1], sel[:], plat[:, c0:c1],
                                 start=True, stop=False)
                nc.tensor.matmul(psum[:, c0:c1], sela[:], paff[:, c0:c1],
                                 start=False, stop=True)

            for t in range(T):
                col = COL_OF[t]
                if t in ACT_SET:
                    w = wap.tile([128, COLS_A], f8, tag="wa")
                    nc.sync.dma_start(w[:], biga_d[col - (NPE - 1)])
                    j = jp.tile([128, LCOL], f32, tag="jlat")
                    nc.scalar.activation(
                        j[:], w[:, 0:LCOL], AF.Copy,
                        scale=possb[:, col:col + 1],
                        accum_out=plat[:, col:col + 1],
                    )
                    aoff = LCOL
                else:
                    pcol = PSCOL_OF[t]
                    if t == LAST:
                        w = wpp.tile([128, LPAD], f8, tag="wlast")
                        nc.sync.dma_start(w[:], bigp_d[pcol, :, 0:LPAD])
                    else:
                        w = wpp.tile([128, COLS_P], f8, tag="wp")
                        nc.sync.dma_start(w[:], bigp_d[pcol])
                    for k in range(KC):
                        nc.tensor.matmul(
                            pslat[:, pcol:pcol + 1],
                            w[:, 128 * k:128 * (k + 1)], ones[:],
                            start=(k == 0), stop=(k == KC - 1),
                        )
                    aoff = LPAD
                if t == LAST:
                    wa_ap, pq_ap = wlaff[:, 0:FA], wlaff[:, FA:2 * FA]
                else:
                    wa_ap = w[:, aoff:aoff + FA]
                    pq_ap = w[:, aoff + FA:aoff + 2 * FA]
                ja = jp.tile([128, FA], bf16, tag="jaff")
                nc.vector.scalar_tensor_tensor(
                    ja[:], wa_ap, 1.0, pq_ap,
                    op0=AL.mult, op1=AL.mult,
                    accum_out=paff[:, col:col + 1],
                )
                if t == 10:
                    # last group's small afferent piece, pulled mid-stream so
                    # only its lateral gates the drain
                    nc.sync.dma_start(wlaff[:], bigp_d[NPE - 1, :, LPAD:COLS_P])
                # early blocks (emission order = scheduler priority)
                if t == ACT_SET[-1]:
                    chansum(NPE - 1, T - 1)              # ACT cols 15..19
                elif t == PE_GROUPS[-2]:
                    # PE cols 0..14: apply 0.9*prev/64, sum, relu, write out
                    nc.vector.tensor_mul(plat[:, 0:NPE - 1],
                                         pslat[:, 0:NPE - 1],
                                         possb[:, 0:NPE - 1])
                    chansum(0, NPE - 1)
                    nc.vector.tensor_scalar_max(res[:, 0:T - 1],
                                                psum[:, 0:T - 1], 0.0)
                    # scalar-engine HWDGE ring keeps the sync FIFO clear
                    nc.scalar.dma_start(out_d[:, 0:T - 1], res[:, 0:T - 1])

            # final column: only the last group's data gates this
            nc.vector.tensor_mul(plat[:, T - 1:T], pslat[:, NPE - 1:NPE],
                                 possb[:, T - 1:T])
            chansum(T - 1, T)
            nc.vector.tensor_scalar_max(res[:, T - 1:T], psum[:, T - 1:T], 0.0)
            nc.sync.dma_start(out_d[:, T - 1:T], res[:, T - 1:T])

    nc.compile()
    return nc


def _get_program():
    if "nc" not in _PROGRAM_CACHE:
        _PROGRAM_CACHE["nc"] = _build_program()
    return _PROGRAM_CACHE["nc"]


def _f8(v):
    return np.clip(v, -240.0, 240.0).astype(F8)


def _ed_rows(w, chunk):
    """fp8 quantize along the last axis with error-diffusion so each
    chunk's sum is preserved to ~one fp8 step."""
    r, n = w.shape
    wv = w.reshape(r * (n // chunk), chunk)
    q = np.empty(wv.shape, F8)
    carry = np.zeros(wv.shape[0], np.float32)
    for k in range(chunk):
        t = wv[:, k] + carry
        qk = _f8(t)
        q[:, k] = qk
        carry = t - qk.astype(np.float32)
    return q.reshape(r, n)


def _gptq_wa(wa_s, pq, t_s):
    """fp8-round scaled afferent weights with the running product-sum
    carried against the fp8 patches, so sum(q*pq) tracks sum(t_s)."""
    r, n = wa_s.shape
    pqf = pq.astype(np.float32)
    q = np.empty((r, n), F8)
    carry = np.zeros(r, np.float32)
    for k in range(n):
        tk = t_s[:, k] + carry
        pk = pqf[:, k]
        safe = np.where(pk == 0, 1.0, pk)
        v = np.where(pk != 0, tk / safe, wa_s[:, k])
        qk = _f8(v)
        q[:, k] = qk
        carry = tk - qk.astype(np.float32) * pk
    return q


def _prep_in_maps(inputs):
    x = np.asarray(inputs["x"], dtype=np.float32)
    prev = np.asarray(inputs["prev_activity"], dtype=np.float32).reshape(C, UNITS)
    wa = np.asarray(inputs["afferent_weights"], dtype=np.float32).reshape(C, UNITS, FA)
    we = np.asarray(inputs["ex_lateral_weights"], dtype=np.float32).reshape(C, UNITS, FW)
    wi = np.asarray(inputs["in_lateral_weights"], dtype=np.float32).reshape(C, UNITS, FW)
    rx = np.asarray(inputs["rx"]).astype(np.int64)
    ry = np.asarray(inputs["ry"]).astype(np.int64)

    u = np.arange(RF)
    ix = rx[:, None] + u                     # [GX, RF]
    iy = ry[:, None] + u                     # [GY, RF]
    px = x[:, ix, :]                         # [C, GX, RF, IMG]
    patches = px[:, :, :, iy]                # [C, GX, RF, GY, RF]
    patches = np.ascontiguousarray(patches.transpose(0, 1, 3, 2, 4))
    patches = patches.reshape(C * UNITS, FA)

    lat = np.concatenate([we, -wi], axis=2).reshape(C * UNITS, LCOL)
    lat_q = _ed_rows(lat * WSCALE, 324)                       # [C*U, 2592] f8
    pq = _f8(patches * PSCALE)                                # [C*U, 576] f8
    wa2 = wa.reshape(C * UNITS, FA)
    t_s = (wa2 * patches) * (WSCALE * PSCALE)
    wa_q = _gptq_wa(wa2 * WSCALE, pq, t_s)                    # [C*U, 576] f8

    lat_q = lat_q.reshape(C, UNITS, LCOL)
    affcat = np.concatenate(
        [wa_q.reshape(C, UNITS, FA), pq.reshape(C, UNITS, FA)], axis=2
    )                                                          # [C, U, 1152]
    prevf = prev * (GAMMA / WSCALE)

    sel = (np.arange(128)[:, None] % S == np.arange(S)[None, :]).astype(np.float32)
    sela = sel * np.float32(1.0 / (WSCALE * PSCALE))

    in_maps = []
    for kcore in range(N_CORES):
        n0 = kcore * PER_CORE
        lq = np.zeros((C, PADU, LCOL), F8)
        lq[:, :PER_CORE] = lat_q[:, n0:n0 + PER_CORE]
        af = np.zeros((C, PADU, 2 * FA), F8)
        af[:, :PER_CORE] = affcat[:, n0:n0 + PER_CORE]
        # partition-major [T, 128, .] with row p = c*S + s
        lqg = lq.reshape(C, T, S, LCOL).transpose(1, 0, 2, 3).reshape(T, 128, LCOL)
        afg = af.reshape(C, T, S, 2 * FA).transpose(1, 0, 2, 3).reshape(T, 128, 2 * FA)

        bigp = np.zeros((NPE, 128, COLS_P), F8)
        biga = np.zeros((T - NPE, 128, COLS_A), F8)
        for t in range(T):
            if t in ACT_SET:
                ai = COL_OF[t] - (NPE - 1)
                biga[ai, :, 0:LCOL] = lqg[t]
                biga[ai, :, LCOL:COLS_A] = afg[t]
            else:
                # transpose lateral: tile[p, 128k+f] = lat[f, 128k+p], 0-pad
                lt = np.zeros((128, LPAD), F8)
                lt[:, 0:LCOL] = lqg[t]
                pi = PSCOL_OF[t]
                bigp[pi, :, 0:LPAD] = (
                    lt.reshape(128, KC, 128).transpose(2, 1, 0).reshape(128, LPAD)
                )
                bigp[pi, :, LPAD:COLS_P] = afg[t]

        pv = np.zeros((C, PADU), np.float32)
        pv[:, :PER_CORE] = prevf[:, n0:n0 + PER_CORE]
        pv = pv.reshape(C, T, S).transpose(0, 2, 1).reshape(128, T)
        pvp = np.empty_like(pv)
        for t in range(T):
            pvp[:, COL_OF[t]] = pv[:, t]
        in_maps.append({
            "bigp": np.ascontiguousarray(bigp),
            "biga": np.ascontiguousarray(biga),
            "possb": np.ascontiguousarray(pvp),
            "sel": sel,
            "sela": sela,
        })
    return in_maps


def _assemble_output(results):
    act = np.empty(UNITS, np.float32)
    for kcore in range(N_CORES):
        o = np.asarray(results[kcore]["out"])            # [S, T] permuted cols
        for t in range(T):
            n0 = kcore * PER_CORE + t * S
            lim = (kcore + 1) * PER_CORE
            if n0 >= lim:
                break
            nn = min(S, lim - n0)
            act[n0:n0 + nn] = o[:nn, COL_OF[t]]
    out = np.broadcast_to(act.reshape(1, GX, GY), (C, GX, GY))
    return np.ascontiguousarray(out, dtype=np.float32)


def kernel(**inputs):
    nc = _get_program()
    in_maps = _prep_in_maps(inputs)
    res = run_bass_kernel_spmd(nc, in_maps, core_ids=list(range(N_CORES)))
    return _assemble_output(res.results)


# revision 18
# speedup vs baseline: 1.0657x; 1.0657x over previous
"""Trainium2 Bass kernel for nn_CortexNetwork (dense_cnn, memory-bound).

Reference computation:
    patches[c,i,j,u,v] = x[c, rx[i]+u, ry[j]+v]
    aff[i,j] = sum_{c,u,v} patches * Wa
    exc[i,j] = sum_c prev[c,i,j] * sum_{x,y} We[c,i,j,x,y]   (inh likewise, Wi)
    out      = broadcast_c(relu(aff + 0.9*exc - 0.9*inh))

Strategy: tensor-parallel over the 36x36=1296 grid units, 162 units per
core on 8 cores (padded to 168 = 21 groups of 8 so every tile is a full
128 partitions = 16 channels x 8 units); every reduction is unit-local
so there are no collectives.  The kernel is HBM-bandwidth-bound, so all
streamed data is fp8_e4m3, quantized on the host with sum-preserving
rounding:

  * lateral We|-Wi rows (x64 scale) use error-diffusion rounding along
    each row, so the device's row sums match the f32 sums to ~1e-3;
  * afferent weights (x64) are rounded with the running product-sum
    carried against the fp8 patches (x16), GPTQ-style, so the device's
    dot products track the f32 products; patches are plain RTN fp8.

Measured end-to-end max-rel-error on the reference inputs is ~1.8e-3
(vs the 2e-2 gate) at 1/4 the f32 bytes.

The 2592-wide lateral row sums are split across three engines so no
engine exceeds the ~30us DMA stream time.  16 groups go to the tensor
engine: their lateral columns are host-transposed (zero-padded to
21x128) into 128x128 LDWEIGHTS blocks consumed as chained matmuls
against a constant fp8 ones vector, accumulating each group's
per-(c,unit) row sum into one PSUM column.  5 groups go to ScalarE as
activation(Copy, scale=0.9*prev/64, accum_out).  VectorE runs the fused
afferent multiply-reduce (scalar_tensor_tensor) per group, the
0.9*prev/64 multiply for the PE block, and the final relu.  The
16-channel sums are 0/1-selector matmuls on PE (the afferent selector
carries the 1/1024 dequant scale).

Tail packing: the last-streamed group owns output column 20 alone and
its record is sent as two DMAs (afferent first); every other column's
channel sums, relu and output DMA (on the scalar-engine HWDGE ring, so
the sync FIFO stays clear) are emitted right after their dependencies
mid-loop, so after the final DMA lands only one group's matmuls, one
merge, one 1-column channel sum and a 32-byte output write remain.
"""

import numpy as np
import ml_dtypes

import concourse.bass as bass
import concourse.bacc as bacc
import concourse.mybir as mybir
from concourse import tile
from concourse.bass_utils import run_bass_kernel_spmd

N_CORES = 8
C = 16
GX = GY = 36
RF = 24
IMG = 64
GAMMA = 0.9

UNITS = GX * GY                  # 1296
PER_CORE = UNITS // N_CORES      # 162
S = 8                            # units per group (partition dim C*S=128)
T = 21                           # groups per core (168 units, 6 padded)
PADU = T * S                     # 168
FW = GX * GY                     # lateral free size per channel: 1296
FA = RF * RF                     # afferent free size per channel: 576
LCOL = 2 * FW                    # 2592 lateral elems per (c,unit)
KC = 21                          # 128-chunks of the padded lateral dim
LPAD = KC * 128                  # 2688
COLS_A = LCOL + 2 * FA           # 3744  (ScalarE-group record)
COLS_P = LPAD + 2 * FA           # 3840  (PE-group record, lateral transposed)
WSCALE = 64.0                    # fp8 scale for We/Wi/Wa
PSCALE = 16.0                    # fp8 scale for patches
ACT_SET = (2, 6, 10, 14, 16)     # groups whose lateral runs on ScalarE
PE_GROUPS = tuple(t for t in range(T) if t not in ACT_SET)
NPE = len(PE_GROUPS)             # 16
LAST = PE_GROUPS[-1]             # 20
# output columns: PE groups except the last at 0..14, ACT at 15..19, and
# the last group alone at 20 so only that column waits on the final DMA
COL_OF = {t: i for i, t in enumerate(PE_GROUPS[:-1])}
COL_OF.update({t: NPE - 1 + j for j, t in enumerate(ACT_SET)})
COL_OF[LAST] = T - 1
PSCOL_OF = {t: i for i, t in enumerate(PE_GROUPS)}   # pslat column

F8 = ml_dtypes.float8_e4m3

_PROGRAM_CACHE = {}


def _build_program():
    f32 = mybir.dt.float32
    f8 = mybir.dt.float8e4
    bf16 = mybir.dt.bfloat16
    AL = mybir.AluOpType
    AF = mybir.ActivationFunctionType

    nc = bacc.Bacc(
        "TRN2", target_bir_lowering=False, debug=False, num_devices=N_CORES
    )
    bigp_d = nc.dram_tensor("bigp", [NPE, 128, COLS_P], f8, kind="ExternalInput").ap()
    biga_d = nc.dram_tensor("biga", [T - NPE, 128, COLS_A], f8, kind="ExternalInput").ap()
    possb_d = nc.dram_tensor("possb", [128, T], f32, kind="ExternalInput").ap()
    sel_d = nc.dram_tensor("sel", [128, S], f32, kind="ExternalInput").ap()
    sela_d = nc.dram_tensor("sela", [128, S], f32, kind="ExternalInput").ap()
    out_d = nc.dram_tensor("out", [S, T], f32, kind="ExternalOutput").ap()

    with tile.TileContext(nc) as tc:
        with (
            tc.tile_pool(name="wp", bufs=8) as wpp,
            tc.tile_pool(name="wa", bufs=4) as wap,
            tc.tile_pool(name="cst", bufs=1) as cp,
            tc.tile_pool(name="junk", bufs=3) as jp,
            tc.tile_pool(name="fin", bufs=1) as fp,
            tc.tile_pool(name="ps", bufs=1, space="PSUM") as pp,
        ):
            possb = cp.tile([128, T], f32, tag="possb")
            sel = cp.tile([128, S], f32, tag="sel")
            sela = cp.tile([128, S], f32, tag="sela")
            ones = cp.tile([128, 1], f8, tag="ones")
            plat = cp.tile([128, T], f32, tag="plat")
            paff = cp.tile([128, T], f32, tag="paff")
            nc.gpsimd.dma_start(possb[:], possb_d[:])
            nc.gpsimd.dma_start(sel[:], sel_d[:])
            nc.gpsimd.dma_start(sela[:], sela_d[:])
            nc.vector.memset(ones[:], 1.0)

            wlaff = cp.tile([128, 2 * FA], f8, tag="wlaff")

            pslat = pp.tile([128, NPE], f32, tag="pslat")
            psum = pp.tile([S, T], f32, tag="ps")
            res = fp.tile([S, T], f32, tag="res")

            def chansum(c0, c1):
                nc.tensor.matmul(psum[:, c0:c1], sel[:], plat[:, c0:c1],
                                 start=True, stop=False)
                nc.tensor.matmul(psum[:, c0:c1], sela[:], paff[:, c0:c1],
                                 start=False, stop=True)

            for t in range(T):
                col = COL_OF[t]
                if t in ACT_SET:
                    w = wap.tile([128, COLS_A], f8, tag="wa")
                    # scalar-ring DMA: second HWDGE FIFO rides through
                    # bubbles on the sync ring, and the activation that
                    # consumes it is on this engine anyway
                    nc.scalar.dma_start(w[:], biga_d[col - (NPE - 1)])
                    j = jp.tile([128, LCOL], f32, tag="jlat")
                    nc.scalar.activation(
                        j[:], w[:, 0:LCOL], AF.Copy,
                        scale=possb[:, col:col + 1],
                        accum_out=plat[:, col:col + 1],
                    )
                    aoff = LCOL
                else:
                    pcol = PSCOL_OF[t]
                    if t == LAST:
                        w = wpp.tile([128, LPAD], f8, tag="wlast")
                        nc.sync.dma_start(w[:, 0:18 * 128],
                                          bigp_d[pcol, :, 0:18 * 128])
                        nc.sync.dma_start(w[:, 18 * 128:LPAD],
                                          bigp_d[pcol, :, 18 * 128:LPAD])
                    else:
                        w = wpp.tile([128, COLS_P], f8, tag="wp")
                        nc.sync.dma_start(w[:], bigp_d[pcol])
                    for k in range(KC):
                        nc.tensor.matmul(
                            pslat[:, pcol:pcol + 1],
                            w[:, 128 * k:128 * (k + 1)], ones[:],
                            start=(k == 0), stop=(k == KC - 1),
                        )
                    aoff = LPAD
                if t == LAST:
                    wa_ap, pq_ap = wlaff[:, 0:FA], wlaff[:, FA:2 * FA]
                else:
                    wa_ap = w[:, aoff:aoff + FA]
                    pq_ap = w[:, aoff + FA:aoff + 2 * FA]
                ja = jp.tile([128, FA], bf16, tag="jaff")
                nc.vector.scalar_tensor_tensor(
                    ja[:], wa_ap, 1.0, pq_ap,
                    op0=AL.mult, op1=AL.mult,
                    accum_out=paff[:, col:col + 1],
                )
                if t == 10:
                    # last group's small afferent piece, pulled mid-stream so
                    # only its lateral gates the drain
                    nc.sync.dma_start(wlaff[:], bigp_d[NPE - 1, :, LPAD:COLS_P])
                # early blocks (emission order = scheduler priority)
                if t == ACT_SET[-1]:
                    chansum(NPE - 1, T - 1)              # ACT cols 15..19
                elif t == PE_GROUPS[-2]:
                    # PE cols 0..14: apply 0.9*prev/64, sum, relu, write out
                    nc.vector.tensor_mul(plat[:, 0:NPE - 1],
                                         pslat[:, 0:NPE - 1],
                                         possb[:, 0:NPE - 1])
                    chansum(0, NPE - 1)
                    nc.vector.tensor_scalar_max(res[:, 0:T - 1],
                                                psum[:, 0:T - 1], 0.0)
                    # scalar-engine HWDGE ring keeps the sync FIFO clear
                    nc.scalar.dma_start(out_d[:, 0:T - 1], res[:, 0:T - 1])

            # final column: only the last group's data gates this
            nc.vector.tensor_mul(plat[:, T - 1:T], pslat[:, NPE - 1:NPE],
                                 possb[:, T - 1:T])
            chansum(T - 1, T)
            nc.vector.tensor_scalar_max(res[:, T - 1:T], psum[:, T - 1:T], 0.0)
            nc.sync.dma_start(out_d[:, T - 1:T], res[:, T - 1:T])

    nc.compile()
    return nc


def _get_program():
    if "nc" not in _PROGRAM_CACHE:
        _PROGRAM_CACHE["nc"] = _build_program()
    return _PROGRAM_CACHE["nc"]


def _f8(v):
    return np.clip(v, -240.0, 240.0).astype(F8)


def _ed_rows(w, chunk):
    """fp8 quantize along the last axis with error-diffusion so each
    chunk's sum is preserved to ~one fp8 step."""
    r, n = w.shape
    wv = w.reshape(r * (n // chunk), chunk)
    q = np.empty(wv.shape, F8)
    carry = np.zeros(wv.shape[0], np.float32)
    for k in range(chunk):
        t = wv[:, k] + carry
        qk = _f8(t)
        q[:, k] = qk
        carry = t - qk.astype(np.float32)
    return q.reshape(r, n)


def _gptq_wa(wa_s, pq, t_s):
    """fp8-round scaled afferent weights with the running product-sum
    carried against the fp8 patches, so sum(q*pq) tracks sum(t_s)."""
    r, n = wa_s.shape
    pqf = pq.astype(np.float32)
    q = np.empty((r, n), F8)
    carry = np.zeros(r, np.float32)
    for k in range(n):
        tk = t_s[:, k] + carry
        pk = pqf[:, k]
        safe = np.where(pk == 0, 1.0, pk)
        v = np.where(pk != 0, tk / safe, wa_s[:, k])
        qk = _f8(v)
        q[:, k] = qk
        carry = tk - qk.astype(np.float32) * pk
    return q


def _prep_in_maps(inputs):
    x = np.asarray(inputs["x"], dtype=np.float32)
    prev = np.asarray(inputs["prev_activity"], dtype=np.float32).reshape(C, UNITS)
    wa = np.asarray(inputs["afferent_weights"], dtype=np.float32).reshape(C, UNITS, FA)
    we = np.asarray(inputs["ex_lateral_weights"], dtype=np.float32).reshape(C, UNITS, FW)
    wi = np.asarray(inputs["in_lateral_weights"], dtype=np.float32).reshape(C, UNITS, FW)
    rx = np.asarray(inputs["rx"]).astype(np.int64)
    ry = np.asarray(inputs["ry"]).astype(np.int64)

    u = np.arange(RF)
    ix = rx[:, None] + u                     # [GX, RF]
    iy = ry[:, None] + u                     # [GY, RF]
    px = x[:, ix, :]                         # [C, GX, RF, IMG]
    patches = px[:, :, :, iy]                # [C, GX, RF, GY, RF]
    patches = np.ascontiguousarray(patches.transpose(0, 1, 3, 2, 4))
    patches = patches.reshape(C * UNITS, FA)

    lat = np.concatenate([we, -wi], axis=2).reshape(C * UNITS, LCOL)
    lat_q = _ed_rows(lat * WSCALE, 324)                       # [C*U, 2592] f8
    pq = _f8(patches * PSCALE)                                # [C*U, 576] f8
    wa2 = wa.reshape(C * UNITS, FA)
    t_s = (wa2 * patches) * (WSCALE * PSCALE)
    wa_q = _gptq_wa(wa2 * WSCALE, pq, t_s)                    # [C*U, 576] f8

    lat_q = lat_q.reshape(C, UNITS, LCOL)
    affcat = np.concatenate(
        [wa_q.reshape(C, UNITS, FA), pq.reshape(C, UNITS, FA)], axis=2
    )                                                          # [C, U, 1152]
    prevf = prev * (GAMMA / WSCALE)

    sel = (np.arange(128)[:, None] % S == np.arange(S)[None, :]).astype(np.float32)
    sela = sel * np.float32(1.0 / (WSCALE * PSCALE))

    in_maps = []
    for kcore in range(N_CORES):
        n0 = kcore * PER_CORE
        lq = np.zeros((C, PADU, LCOL), F8)
        lq[:, :PER_CORE] = lat_q[:, n0:n0 + PER_CORE]
        af = np.zeros((C, PADU, 2 * FA), F8)
        af[:, :PER_CORE] = affcat[:, n0:n0 + PER_CORE]
        # partition-major [T, 128, .] with row p = c*S + s
        lqg = lq.reshape(C, T, S, LCOL).transpose(1, 0, 2, 3).reshape(T, 128, LCOL)
        afg = af.reshape(C, T, S, 2 * FA).transpose(1, 0, 2, 3).reshape(T, 128, 2 * FA)

        bigp = np.zeros((NPE, 128, COLS_P), F8)
        biga = np.zeros((T - NPE, 128, COLS_A), F8)
        for t in range(T):
            if t in ACT_SET:
                ai = COL_OF[t] - (NPE - 1)
                biga[ai, :, 0:LCOL] = lqg[t]
                biga[ai, :, LCOL:COLS_A] = afg[t]
            else:
                # transpose lateral: tile[p, 128k+f] = lat[f, 128k+p], 0-pad
                lt = np.zeros((128, LPAD), F8)
                lt[:, 0:LCOL] = lqg[t]
                pi = PSCOL_OF[t]
                bigp[pi, :, 0:LPAD] = (
                    lt.reshape(128, KC, 128).transpose(2, 1, 0).reshape(128, LPAD)
                )
                bigp[pi, :, LPAD:COLS_P] = afg[t]

        pv = np.zeros((C, PADU), np.float32)
        pv[:, :PER_CORE] = prevf[:, n0:n0 + PER_CORE]
        pv = pv.reshape(C, T, S).transpose(0, 2, 1).reshape(128, T)
        pvp = np.empty_like(pv)
        for t in range(T):
            pvp[:, COL_OF[t]] = pv[:, t]
        in_maps.append({
            "bigp": np.ascontiguousarray(bigp),
            "biga": np.ascontiguousarray(biga),
            "possb": np.ascontiguousarray(pvp),
            "sel": sel,
            "sela": sela,
        })
    return in_maps


def _assemble_output(results):
    act = np.empty(UNITS, np.float32)
    for kcore in range(N_CORES):
        o = np.asarray(results[kcore]["out"])            # [S, T] permuted cols
        for t in range(T):
            n0 = kcore * PER_CORE + t * S
            lim = (kcore + 1) * PER_CORE
            if n0 >= lim:
                break
            nn = min(S, lim - n0)
            act[n0:n0 + nn] = o[:nn, COL_OF[t]]
    out = np.broadcast_to(act.reshape(1, GX, GY), (C, GX, GY))
    return np.ascontiguousarray(out, dtype=np.float32)


def kernel(**inputs):
    nc = _get_program()
    in_maps = _prep_in_maps(inputs)
    res = run_bass_kernel_spmd(nc, in_maps, core_ids=list(range(N_CORES)))
    return _assemble_output(res.results)
